# revision 2
# baseline (speedup 1.0000x reference)
"""Multi-head attention with "restricted softmax" on 8 TRN2 NeuronCores.

Reference computation (per head):
    score = Q @ K.T / sqrt(D)                       # [S, S]
    attn  = exp(score) / (1 + sum_k exp(score))     # restricted softmax
    out   = attn @ V                                # [S, D]

Full problem: B=2, H=16, S=2048, D=64  ->  32 heads, 4 heads per core.

Design (vs the padded-K / ACT-only-exp baseline; ~148us vs ~213us traced):
  - Host prep: Q^T/K^T pre-transposed to [64, S] fp16 and V with a ones
    column appended ([S, 65] fp16); no on-device staging (no fp32 loads,
    casts, DRAM bounce, or X-bar transposes).
  - Scores contract over the TRUE d=64 via PE row tiling: k-blocks 2i / 2i+1
    run concurrently in array rows 0-63 / 64-127 (tile_position (0,0) /
    (64,0)); Q^T/K^T are DMA'd into both partition halves. Adjacent-emitted
    pair matmuls co-start within 4 ns, ~2x scores throughput vs a
    zero-padded K=128 contraction.
  - exp splits across engines (the ScalarE spline alone runs 1 elem/cycle/
    lane = a ~110us floor): half the k-blocks use the exact ACT exp, half
    run on the DVE as a 1-op Schraudolph: int16 out of
    round(1024*(log2e*s/8 + 15 - 0.0573)) IS the fp16 bit pattern of
    ~exp(s/8) (+-2% ripple); the exact denominators (ones column) cancel
    the common mode, landing ~1e-2 end-to-end vs the 2e-2 gate. Engines are
    assigned per PAIR (both banks of a scores pair free together, enabling
    the co-start); the DVE owns even pairs so ACT is free for the pass-end
    eviction.
  - PV streams et [128, q] against stationary [V|1] (65-col LDWEIGHTS once
    per k-block), accumulating oT[65, q] in PSUM; row 64 = denominator.
  - oT ships raw (ACT evict fp16 -> DMA); normalize + transpose happen on
    the host. Deleting the device epilogue frees PSUM for a 3-deep scores
    rotation (6 banks + 2 oT) that keeps the PE fed.
"""

import os

import numpy as np

import concourse.bass as bass  # noqa: F401  (bass must import before tile)
import concourse.mybir as mybir
import concourse.tile as tile
from concourse import bacc
from concourse.bass_utils import run_bass_kernel_spmd

B, H, S, D = 2, 16, 2048, 64
N_CORES = 8
HPC = (B * H) // N_CORES  # heads per core = 4

F32 = mybir.dt.float32
F16 = mybir.dt.float16
I16 = mybir.dt.int16
EXP = mybir.ActivationFunctionType.Exp
MULT = mybir.AluOpType.mult
ADD = mybir.AluOpType.add

SCALE = 1.0 / 8.0   # 1/sqrt(D)
NQ = S // 128       # 16 tiles of 128 along both q and k
QH = 1024           # q-half width processed per pass
NPAIR = NQ // 2     # row-tiled k-block pairs per pass

# Schraudolph fast-exp constants (fp16 bit trick):
# bitcast_f16(round(s*A + Bc)) ~ exp(s/8), c tuned on the full pipeline
A_FEXP = 1024.0 * 1.4426950408889634 / 8.0
B_FEXP = (15.0 - 0.0573) * 1024.0
DVE_PAIRS = frozenset(i for i in range(NPAIR) if i % 2 == 0)


class _HeadInputs:
    def __init__(self, ctx, h):
        self.ctx = ctx
        self.h = h

    def start_dma(self):
        nc, pools, h = self.ctx["nc"], self.ctx, self.h
        self.qT = pools["qkt_pool"].tile([128, S], F16, tag="qT", name=f"qT{h}")
        self.kT = pools["qkt_pool"].tile([128, S], F16, tag="kT", name=f"kT{h}")
        for half in range(2):
            ps = slice(64 * half, 64 * half + 64)
            nc.sync.dma_start(self.qT[ps, :], pools["qT_dram"][h])
            nc.sync.dma_start(self.kT[ps, :], pools["kT_dram"][h])
        self.v1 = pools["head_pool"].tile([128, NQ, D + 1], F16, tag="v1",
                                          name=f"v1_{h}")
        nc.sync.dma_start(
            self.v1[:], pools["v1_dram"][h].rearrange("(n p) c -> p n c", p=128)
        )


def _attention(tc):
    nc = tc.nc
    qT_dram = nc.dram_tensor("qT", [HPC, 64, S], F16, kind="ExternalInput").ap()
    kT_dram = nc.dram_tensor("kT", [HPC, 64, S], F16, kind="ExternalInput").ap()
    v1_dram = nc.dram_tensor("v1", [HPC, S, D + 1], F16, kind="ExternalInput").ap()
    # transposed-unnormalized output: [d 0..63 | denom] x q, fixed up on host
    oT_dram = nc.dram_tensor("outT", [HPC, 2, D + 1, QH], F16,
                             kind="ExternalOutput").ap()

    with (
        tc.tile_pool(name="head_io", bufs=2) as head_pool,
        tc.tile_pool(name="qkt", bufs=2) as qkt_pool,
        tc.tile_pool(name="et", bufs=4) as et_pool,
        tc.tile_pool(name="epi", bufs=2) as epi_pool,
        tc.tile_pool(name="ps_s", bufs=3, space="PSUM") as ps_s_pool,
        tc.tile_pool(name="ps_o", bufs=1, space="PSUM") as ps_o_pool,
    ):
        pools = {
            "nc": nc, "qT_dram": qT_dram, "kT_dram": kT_dram, "v1_dram": v1_dram,
            "head_pool": head_pool, "qkt_pool": qkt_pool,
        }

        heads = [_HeadInputs(pools, h) for h in range(HPC)]
        heads[0].start_dma()

        def _sc_mm(hd, qh, i, s_ps, lo, b):
            """One scores matmul: k-block 2i+lo in array rows [64*lo, +64)."""
            q0 = qh * QH + b * 512
            nc.tensor.matmul(
                s_ps[:, b * 512:(b + 1) * 512],
                hd.kT[64 * lo:64 * lo + 64,
                      (2 * i + lo) * 128:(2 * i + lo + 1) * 128],
                hd.qT[64 * lo:64 * lo + 64, q0:q0 + 512],
                start=True, stop=True, tile_position=(64 * lo, 0),
            )

        def emit_scores_pair(hd, qh, i, sA, sB):
            """k-blocks 2i (rows 0-63) and 2i+1 (rows 64-127), concurrent."""
            for b in range(2):
                _sc_mm(hd, qh, i, sA, 0, b)
                _sc_mm(hd, qh, i, sB, 1, b)

        def emit_exp(i, k, s_ps):
            if i not in DVE_PAIRS:
                et = et_pool.tile([128, QH], F16, tag="et", name=f"et{k}")
                nc.scalar.activation(et[:], s_ps[:], EXP, scale=SCALE)
                return et[:]
            et = et_pool.tile([128, QH], I16, tag="et", name=f"et{k}")
            nc.vector.tensor_scalar(
                out=et[:], in0=s_ps[:], scalar1=A_FEXP, scalar2=B_FEXP,
                op0=MULT, op1=ADD,
            )
            return et[:].bitcast(F16)

        def emit_pv(hd, oT, k, et):
            for b in range(2):
                nc.tensor.matmul(
                    oT[:, b * 512:(b + 1) * 512],
                    hd.v1[:, k, :],
                    et[:, b * 512:(b + 1) * 512],
                    start=(k == 0), stop=(k == NQ - 1),
                )

        passes = [(h, qh) for h in range(HPC) for qh in range(S // QH)]
        s_carry = None
        for idx, (h, qh) in enumerate(passes):
            hd = heads[h]
            if qh == 0 and h + 1 < HPC:
                heads[h + 1].start_dma()

            oT = ps_o_pool.tile([D + 1, QH], F32, tag="oT", name="oT")
            for i in range(NPAIR):
                if i == 0 and s_carry is not None:
                    sA, sB = s_carry
                    s_carry = None
                else:
                    sA = ps_s_pool.tile([128, QH], F32, tag="s", name=f"sA{i}")
                    sB = ps_s_pool.tile([128, QH], F32, tag="s", name=f"sB{i}")
                    emit_scores_pair(hd, qh, i, sA, sB)
                etA = emit_exp(i, 2 * i, sA)
                etB = emit_exp(i, 2 * i + 1, sB)
                emit_pv(hd, oT, 2 * i, etA)
                emit_pv(hd, oT, 2 * i + 1, etB)
                del sA, sB, etA, etB
            # hoist the next pass's first scores pair into this pass's tail
            if idx + 1 < len(passes):
                nh, nqh = passes[idx + 1]
                sA = ps_s_pool.tile([128, QH], F32, tag="s", name="scA")
                sB = ps_s_pool.tile([128, QH], F32, tag="s", name="scB")
                emit_scores_pair(heads[nh], nqh, 0, sA, sB)
                s_carry = (sA, sB)
            # DMA can't source PSUM: ACT (the PSUM-near engine) evicts to fp16
            o16 = epi_pool.tile([D + 1, QH], F16, tag="o16", name="o16")
            nc.scalar.copy(o16[:], oT[:])
            nc.sync.dma_start(oT_dram[h, qh], o16[:])


_NC_CACHE = None
_TRACE_READY = False


def _enable_tracing():
    """Register the NTFF profile hook that this image's antenv lacks, and
    keep profiling artifacts local instead of uploading to a bucket."""
    global _TRACE_READY
    if _TRACE_READY:
        return
    import sys
    import types

    import antenv
    import concourse.bass_utils as bu
    from trn_agent_boot.trn_boot import _ntff_profile_via_ctypes

    if "antenv.axon_hooks" not in sys.modules:
        mod = types.ModuleType("antenv.axon_hooks")
        mod._hook = None

        def set_axon_ntff_profile_hook(h):
            mod._hook = h

        def get_axon_ntff_profile_hook():
            return mod._hook

        mod.set_axon_ntff_profile_hook = set_axon_ntff_profile_hook
        mod.get_axon_ntff_profile_hook = get_axon_ntff_profile_hook
        sys.modules["antenv.axon_hooks"] = mod
        antenv.axon_hooks = mod

    hooks = sys.modules["antenv.axon_hooks"]
    if hooks.get_axon_ntff_profile_hook() is None:
        hooks.set_axon_ntff_profile_hook(
            _ntff_profile_via_ctypes("/opt/axon/libaxon_pjrt.so")
        )
    bu.upload_artifacts = lambda tmpdir: tmpdir
    _TRACE_READY = True


def _build():
    global _NC_CACHE
    if _NC_CACHE is None:
        nc = bacc.Bacc("TRN2", target_bir_lowering=False, debug=False)
        with tile.TileContext(nc) as tc:
            _attention(tc)
        nc.compile()
        _NC_CACHE = nc
    return _NC_CACHE


def _run(query, key, value, trace=False, tmpdir=None):
    if trace:
        _enable_tracing()
    q = np.asarray(query, dtype=np.float32).reshape(B * H, S, D)
    k = np.asarray(key, dtype=np.float32).reshape(B * H, S, D)
    v = np.asarray(value, dtype=np.float32).reshape(B * H, S, D)
    # host-side formatting: fp16 transposes + ones column
    qT = np.ascontiguousarray(q.transpose(0, 2, 1)).astype(np.float16)
    kT = np.ascontiguousarray(k.transpose(0, 2, 1)).astype(np.float16)
    v1 = np.empty((B * H, S, D + 1), dtype=np.float16)
    v1[:, :, :D] = v
    v1[:, :, D] = 1.0
    in_maps = [
        {
            "qT": qT[c * HPC:(c + 1) * HPC],
            "kT": kT[c * HPC:(c + 1) * HPC],
            "v1": v1[c * HPC:(c + 1) * HPC],
        }
        for c in range(N_CORES)
    ]
    nc = _build()
    res = run_bass_kernel_spmd(
        nc, in_maps, core_ids=list(range(N_CORES)), trace=trace, tmpdir=tmpdir
    )
    # host-side normalize + transpose: outT is [HPC, 2, D+1, QH] per core
    oT = np.stack([res.results[c]["outT"] for c in range(N_CORES)])
    oT = oT.reshape(B * H, 2, D + 1, QH).astype(np.float32)
    out = oT[:, :, :D, :] / (1.0 + oT[:, :, D:D + 1, :])
    out = out.transpose(0, 1, 3, 2).reshape(B, H, S, D)
    return np.ascontiguousarray(out), res


def kernel(query, key, value):
    out, _ = _run(query, key, value, trace=bool(int(os.environ.get("BASS_TRACE", "0"))))
    return out


# revision 3
# speedup vs baseline: 1.0222x; 1.0222x over previous
"""Multi-head attention with "restricted softmax" on 8 TRN2 NeuronCores.

Reference computation (per head):
    score = Q @ K.T / sqrt(D)                       # [S, S]
    attn  = exp(score) / (1 + sum_k exp(score))     # restricted softmax
    out   = attn @ V                                # [S, D]

Full problem: B=2, H=16, S=2048, D=64  ->  32 heads, 4 heads per core.

Design (vs the padded-K / ACT-only-exp baseline; ~148us vs ~213us traced):
  - Host prep: Q^T/K^T pre-transposed to [64, S] fp16 and V with a ones
    column appended ([S, 65] fp16); no on-device staging (no fp32 loads,
    casts, DRAM bounce, or X-bar transposes).
  - Scores contract over the TRUE d=64 via PE row tiling: k-blocks 2i / 2i+1
    run concurrently in array rows 0-63 / 64-127 (tile_position (0,0) /
    (64,0)); Q^T/K^T are DMA'd into both partition halves. Adjacent-emitted
    pair matmuls co-start within 4 ns, ~2x scores throughput vs a
    zero-padded K=128 contraction.
  - exp splits across engines (the ScalarE spline alone runs 1 elem/cycle/
    lane = a ~110us floor): half the k-blocks use the exact ACT exp, half
    run on the DVE as a 1-op Schraudolph: int16 out of
    round(1024*(log2e*s/8 + 15 - 0.0573)) IS the fp16 bit pattern of
    ~exp(s/8) (+-2% ripple); the exact denominators (ones column) cancel
    the common mode, landing ~1e-2 end-to-end vs the 2e-2 gate. Engines are
    assigned per PAIR (both banks of a scores pair free together, enabling
    the co-start); the DVE owns even pairs so ACT is free for the pass-end
    eviction.
  - PV streams et [128, q] against stationary [V|1] (65-col LDWEIGHTS once
    per k-block), accumulating oT[65, q] in PSUM; row 64 = denominator.
  - oT ships raw (ACT evict fp16 -> DMA); normalize + transpose happen on
    the host. Deleting the device epilogue frees PSUM for a 3-deep scores
    rotation (6 banks + 2 oT) that keeps the PE fed.
"""

import os

import numpy as np

import concourse.bass as bass  # noqa: F401  (bass must import before tile)
import concourse.mybir as mybir
import concourse.tile as tile
from concourse import bacc
from concourse.bass_utils import run_bass_kernel_spmd

B, H, S, D = 2, 16, 2048, 64
N_CORES = 8
HPC = (B * H) // N_CORES  # heads per core = 4

F32 = mybir.dt.float32
F16 = mybir.dt.float16
I16 = mybir.dt.int16
EXP = mybir.ActivationFunctionType.Exp
MULT = mybir.AluOpType.mult
ADD = mybir.AluOpType.add

SCALE = 1.0 / 8.0   # 1/sqrt(D)
NQ = S // 128       # 16 tiles of 128 along both q and k
QH = 1024           # q-half width processed per pass
NPAIR = NQ // 2     # row-tiled k-block pairs per pass

# Schraudolph fast-exp constants (fp16 bit trick):
# bitcast_f16(round(s*A + Bc)) ~ exp(s/8), c tuned on the full pipeline
A_FEXP = 1024.0 * 1.4426950408889634 / 8.0
B_FEXP = (15.0 - 0.0573) * 1024.0
DVE_PAIRS = frozenset(i for i in range(NPAIR) if i % 2 == 0)


class _HeadInputs:
    def __init__(self, ctx, h):
        self.ctx = ctx
        self.h = h

    def start_dma(self):
        nc, pools, h = self.ctx["nc"], self.ctx, self.h
        self.qT = pools["qkt_pool"].tile([128, S], F16, tag="qT", name=f"qT{h}")
        self.kT = pools["qkt_pool"].tile([128, S], F16, tag="kT", name=f"kT{h}")
        for half in range(2):
            ps = slice(64 * half, 64 * half + 64)
            nc.sync.dma_start(self.qT[ps, :], pools["qT_dram"][h])
            nc.sync.dma_start(self.kT[ps, :], pools["kT_dram"][h])
        self.v1 = pools["head_pool"].tile([128, NQ, D + 1], F16, tag="v1",
                                          name=f"v1_{h}")
        nc.sync.dma_start(
            self.v1[:], pools["v1_dram"][h].rearrange("(n p) c -> p n c", p=128)
        )


def _attention(tc):
    nc = tc.nc
    qT_dram = nc.dram_tensor("qT", [HPC, 64, S], F16, kind="ExternalInput").ap()
    kT_dram = nc.dram_tensor("kT", [HPC, 64, S], F16, kind="ExternalInput").ap()
    v1_dram = nc.dram_tensor("v1", [HPC, S, D + 1], F16, kind="ExternalInput").ap()
    # transposed-unnormalized output: [d 0..63 | denom] x q, fixed up on host
    oT_dram = nc.dram_tensor("outT", [HPC, 2, D + 1, QH], F16,
                             kind="ExternalOutput").ap()

    with (
        tc.tile_pool(name="head_io", bufs=2) as head_pool,
        tc.tile_pool(name="qkt", bufs=2) as qkt_pool,
        tc.tile_pool(name="et", bufs=4) as et_pool,
        tc.tile_pool(name="epi", bufs=2) as epi_pool,
        tc.tile_pool(name="ps_s", bufs=3, space="PSUM") as ps_s_pool,
        tc.tile_pool(name="ps_o", bufs=1, space="PSUM") as ps_o_pool,
    ):
        pools = {
            "nc": nc, "qT_dram": qT_dram, "kT_dram": kT_dram, "v1_dram": v1_dram,
            "head_pool": head_pool, "qkt_pool": qkt_pool,
        }

        heads = [_HeadInputs(pools, h) for h in range(HPC)]
        heads[0].start_dma()

        def _sc_mm(hd, qh, i, s_ps, lo, b):
            """One scores matmul: k-block 2i+lo in array rows [64*lo, +64)."""
            q0 = qh * QH + b * 512
            nc.tensor.matmul(
                s_ps[:, b * 512:(b + 1) * 512],
                hd.kT[64 * lo:64 * lo + 64,
                      (2 * i + lo) * 128:(2 * i + lo + 1) * 128],
                hd.qT[64 * lo:64 * lo + 64, q0:q0 + 512],
                start=True, stop=True, tile_position=(64 * lo, 0),
            )

        def emit_scores_pair(hd, qh, i, sA, sB):
            """k-blocks 2i (rows 0-63) and 2i+1 (rows 64-127), concurrent."""
            for b in range(2):
                _sc_mm(hd, qh, i, sA, 0, b)
                _sc_mm(hd, qh, i, sB, 1, b)

        def emit_exp(i, k, s_ps):
            if i not in DVE_PAIRS:
                et = et_pool.tile([128, QH], F16, tag="et", name=f"et{k}")
                nc.scalar.activation(et[:], s_ps[:], EXP, scale=SCALE)
                return et[:]
            et = et_pool.tile([128, QH], I16, tag="et", name=f"et{k}")
            nc.vector.tensor_scalar(
                out=et[:], in0=s_ps[:], scalar1=A_FEXP, scalar2=B_FEXP,
                op0=MULT, op1=ADD,
            )
            return et[:].bitcast(F16)

        def emit_pv(hd, oT, k, et):
            for b in range(2):
                nc.tensor.matmul(
                    oT[:, b * 512:(b + 1) * 512],
                    hd.v1[:, k, :],
                    et[:, b * 512:(b + 1) * 512],
                    start=(k == 0), stop=(k == NQ - 1),
                )

        passes = [(h, qh) for h in range(HPC) for qh in range(S // QH)]
        s_carry = None
        for idx, (h, qh) in enumerate(passes):
            hd = heads[h]
            if qh == 0 and h + 1 < HPC:
                heads[h + 1].start_dma()

            oT = ps_o_pool.tile([D + 1, QH], F32, tag="oT", name="oT")
            pv_q = []
            for i in range(NPAIR):
                if i == 0 and s_carry is not None:
                    sA, sB = s_carry
                    s_carry = None
                else:
                    sA = ps_s_pool.tile([128, QH], F32, tag="s", name=f"sA{i}")
                    sB = ps_s_pool.tile([128, QH], F32, tag="s", name=f"sB{i}")
                    emit_scores_pair(hd, qh, i, sA, sB)
                etA = emit_exp(i, 2 * i, sA)
                etB = emit_exp(i, 2 * i + 1, sB)
                # pv emission lags one pair so the next scores pair outranks
                # pending pv in scheduler priority (measured ~2-3us better)
                pv_q.append((2 * i, etA))
                pv_q.append((2 * i + 1, etB))
                while len(pv_q) > 2:
                    k_, et_ = pv_q.pop(0)
                    emit_pv(hd, oT, k_, et_)
                del sA, sB, etA, etB
            for k_, et_ in pv_q:
                emit_pv(hd, oT, k_, et_)
            # hoist the next pass's first scores pair into this pass's tail
            if idx + 1 < len(passes):
                nh, nqh = passes[idx + 1]
                sA = ps_s_pool.tile([128, QH], F32, tag="s", name="scA")
                sB = ps_s_pool.tile([128, QH], F32, tag="s", name="scB")
                emit_scores_pair(heads[nh], nqh, 0, sA, sB)
                s_carry = (sA, sB)
            # DMA can't source PSUM: ACT (the PSUM-near engine) evicts to fp16
            o16 = epi_pool.tile([D + 1, QH], F16, tag="o16", name="o16")
            nc.scalar.copy(o16[:], oT[:])
            nc.sync.dma_start(oT_dram[h, qh], o16[:])


_NC_CACHE = None
_TRACE_READY = False


def _enable_tracing():
    """Register the NTFF profile hook that this image's antenv lacks, and
    keep profiling artifacts local instead of uploading to a bucket."""
    global _TRACE_READY
    if _TRACE_READY:
        return
    import sys
    import types

    import antenv
    import concourse.bass_utils as bu
    from trn_agent_boot.trn_boot import _ntff_profile_via_ctypes

    if "antenv.axon_hooks" not in sys.modules:
        mod = types.ModuleType("antenv.axon_hooks")
        mod._hook = None

        def set_axon_ntff_profile_hook(h):
            mod._hook = h

        def get_axon_ntff_profile_hook():
            return mod._hook

        mod.set_axon_ntff_profile_hook = set_axon_ntff_profile_hook
        mod.get_axon_ntff_profile_hook = get_axon_ntff_profile_hook
        sys.modules["antenv.axon_hooks"] = mod
        antenv.axon_hooks = mod

    hooks = sys.modules["antenv.axon_hooks"]
    if hooks.get_axon_ntff_profile_hook() is None:
        hooks.set_axon_ntff_profile_hook(
            _ntff_profile_via_ctypes("/opt/axon/libaxon_pjrt.so")
        )
    bu.upload_artifacts = lambda tmpdir: tmpdir
    _TRACE_READY = True


def _build():
    global _NC_CACHE
    if _NC_CACHE is None:
        nc = bacc.Bacc("TRN2", target_bir_lowering=False, debug=False)
        with tile.TileContext(nc) as tc:
            _attention(tc)
        nc.compile()
        _NC_CACHE = nc
    return _NC_CACHE


def _run(query, key, value, trace=False, tmpdir=None):
    if trace:
        _enable_tracing()
    q = np.asarray(query, dtype=np.float32).reshape(B * H, S, D)
    k = np.asarray(key, dtype=np.float32).reshape(B * H, S, D)
    v = np.asarray(value, dtype=np.float32).reshape(B * H, S, D)
    # host-side formatting: fp16 transposes + ones column
    qT = np.ascontiguousarray(q.transpose(0, 2, 1)).astype(np.float16)
    kT = np.ascontiguousarray(k.transpose(0, 2, 1)).astype(np.float16)
    v1 = np.empty((B * H, S, D + 1), dtype=np.float16)
    v1[:, :, :D] = v
    v1[:, :, D] = 1.0
    in_maps = [
        {
            "qT": qT[c * HPC:(c + 1) * HPC],
            "kT": kT[c * HPC:(c + 1) * HPC],
            "v1": v1[c * HPC:(c + 1) * HPC],
        }
        for c in range(N_CORES)
    ]
    nc = _build()
    res = run_bass_kernel_spmd(
        nc, in_maps, core_ids=list(range(N_CORES)), trace=trace, tmpdir=tmpdir
    )
    # host-side normalize + transpose: outT is [HPC, 2, D+1, QH] per core
    oT = np.stack([res.results[c]["outT"] for c in range(N_CORES)])
    oT = oT.reshape(B * H, 2, D + 1, QH).astype(np.float32)
    out = oT[:, :, :D, :] / (1.0 + oT[:, :, D:D + 1, :])
    out = out.transpose(0, 1, 3, 2).reshape(B, H, S, D)
    return np.ascontiguousarray(out), res


def kernel(query, key, value):
    out, _ = _run(query, key, value, trace=bool(int(os.environ.get("BASS_TRACE", "0"))))
    return out


# revision 5
# speedup vs baseline: 1.0271x; 1.0048x over previous
"""Multi-head attention with "restricted softmax" on 8 TRN2 NeuronCores.

Reference computation (per head):
    score = Q @ K.T / sqrt(D)                       # [S, S]
    attn  = exp(score) / (1 + sum_k exp(score))     # restricted softmax
    out   = attn @ V                                # [S, D]

Full problem: B=2, H=16, S=2048, D=64  ->  32 heads, 4 heads per core.

Design (vs the padded-K / ACT-only-exp baseline; ~148us vs ~213us traced):
  - Host prep: Q^T/K^T pre-transposed to [64, S] fp16 and V with a ones
    column appended ([S, 65] fp16); no on-device staging (no fp32 loads,
    casts, DRAM bounce, or X-bar transposes).
  - Scores contract over the TRUE d=64 via PE row tiling: k-blocks 2i / 2i+1
    run concurrently in array rows 0-63 / 64-127 (tile_position (0,0) /
    (64,0)); Q^T/K^T are DMA'd into both partition halves. Adjacent-emitted
    pair matmuls co-start within 4 ns, ~2x scores throughput vs a
    zero-padded K=128 contraction.
  - exp splits across engines (the ScalarE spline alone runs 1 elem/cycle/
    lane = a ~110us floor): half the k-blocks use the exact ACT exp, half
    run on the DVE as a 1-op Schraudolph: int16 out of
    round(1024*(log2e*s/8 + 15 - 0.0573)) IS the fp16 bit pattern of
    ~exp(s/8) (+-2% ripple); the exact denominators (ones column) cancel
    the common mode, landing ~1e-2 end-to-end vs the 2e-2 gate. Engines are
    assigned per PAIR (both banks of a scores pair free together, enabling
    the co-start); the DVE owns even pairs so ACT is free for the pass-end
    eviction.
  - PV streams et [128, q] against stationary [V|1] (65-col LDWEIGHTS once
    per k-block), accumulating oT[65, q] in PSUM; row 64 = denominator.
  - oT ships raw (ACT evict fp16 -> DMA); normalize + transpose happen on
    the host. Deleting the device epilogue frees PSUM for a 3-deep scores
    rotation (6 banks + 2 oT) that keeps the PE fed.
"""

import os

import numpy as np

import concourse.bass as bass  # noqa: F401  (bass must import before tile)
import concourse.mybir as mybir
import concourse.tile as tile
from concourse import bacc
from concourse.bass_utils import run_bass_kernel_spmd

B, H, S, D = 2, 16, 2048, 64
N_CORES = 8
HPC = (B * H) // N_CORES  # heads per core = 4

F32 = mybir.dt.float32
F16 = mybir.dt.float16
I16 = mybir.dt.int16
EXP = mybir.ActivationFunctionType.Exp
MULT = mybir.AluOpType.mult
ADD = mybir.AluOpType.add

SCALE = 1.0 / 8.0   # 1/sqrt(D)
NQ = S // 128       # 16 tiles of 128 along both q and k
QH = 1024           # q-half width processed per pass
NPAIR = NQ // 2     # row-tiled k-block pairs per pass

# Schraudolph fast-exp constants (fp16 bit trick):
# bitcast_f16(round(s*A + Bc)) ~ exp(s/8), c tuned on the full pipeline
A_FEXP = 1024.0 * 1.4426950408889634 / 8.0
B_FEXP = (15.0 - 0.0573) * 1024.0
# Engine ownership is by PAIR (both banks of a scores pair free together);
# DVE on odd pairs / ACT on even measured ~2us faster than the reverse.
DVE_PAIRS = frozenset(i for i in range(NPAIR) if i % 2 == 1)


class _HeadInputs:
    def __init__(self, ctx, h):
        self.ctx = ctx
        self.h = h

    def start_dma(self):
        nc, pools, h = self.ctx["nc"], self.ctx, self.h
        self.qT = pools["qkt_pool"].tile([128, S], F16, tag="qT", name=f"qT{h}")
        self.kT = pools["qkt_pool"].tile([128, S], F16, tag="kT", name=f"kT{h}")
        for half in range(2):
            ps = slice(64 * half, 64 * half + 64)
            nc.sync.dma_start(self.qT[ps, :], pools["qT_dram"][h])
            nc.sync.dma_start(self.kT[ps, :], pools["kT_dram"][h])
        self.v1 = pools["head_pool"].tile([128, NQ, D + 1], F16, tag="v1",
                                          name=f"v1_{h}")
        nc.sync.dma_start(
            self.v1[:], pools["v1_dram"][h].rearrange("(n p) c -> p n c", p=128)
        )


def _attention(tc):
    nc = tc.nc
    qT_dram = nc.dram_tensor("qT", [HPC, 64, S], F16, kind="ExternalInput").ap()
    kT_dram = nc.dram_tensor("kT", [HPC, 64, S], F16, kind="ExternalInput").ap()
    v1_dram = nc.dram_tensor("v1", [HPC, S, D + 1], F16, kind="ExternalInput").ap()
    # transposed-unnormalized output: [d 0..63 | denom] x q, fixed up on host
    oT_dram = nc.dram_tensor("outT", [HPC, 2, D + 1, QH], F16,
                             kind="ExternalOutput").ap()

    with (
        tc.tile_pool(name="head_io", bufs=2) as head_pool,
        tc.tile_pool(name="qkt", bufs=2) as qkt_pool,
        tc.tile_pool(name="et", bufs=4) as et_pool,
        tc.tile_pool(name="epi", bufs=2) as epi_pool,
        tc.tile_pool(name="ps_s", bufs=3, space="PSUM") as ps_s_pool,
        tc.tile_pool(name="ps_o", bufs=1, space="PSUM") as ps_o_pool,
    ):
        pools = {
            "nc": nc, "qT_dram": qT_dram, "kT_dram": kT_dram, "v1_dram": v1_dram,
            "head_pool": head_pool, "qkt_pool": qkt_pool,
        }

        heads = [_HeadInputs(pools, h) for h in range(HPC)]
        heads[0].start_dma()

        def _sc_mm(hd, qh, i, s_ps, lo, b):
            """One scores matmul: k-block 2i+lo in array rows [64*lo, +64)."""
            q0 = qh * QH + b * 512
            nc.tensor.matmul(
                s_ps[:, b * 512:(b + 1) * 512],
                hd.kT[64 * lo:64 * lo + 64,
                      (2 * i + lo) * 128:(2 * i + lo + 1) * 128],
                hd.qT[64 * lo:64 * lo + 64, q0:q0 + 512],
                start=True, stop=True, tile_position=(64 * lo, 0),
            )

        def emit_scores_pair(hd, qh, i, sA, sB):
            """k-blocks 2i (rows 0-63) and 2i+1 (rows 64-127), concurrent."""
            for b in range(2):
                _sc_mm(hd, qh, i, sA, 0, b)
                _sc_mm(hd, qh, i, sB, 1, b)

        def emit_exp(i, k, s_ps):
            if i not in DVE_PAIRS:
                et = et_pool.tile([128, QH], F16, tag="et", name=f"et{k}")
                nc.scalar.activation(et[:], s_ps[:], EXP, scale=SCALE)
                return et[:]
            et = et_pool.tile([128, QH], I16, tag="et", name=f"et{k}")
            nc.vector.tensor_scalar(
                out=et[:], in0=s_ps[:], scalar1=A_FEXP, scalar2=B_FEXP,
                op0=MULT, op1=ADD,
            )
            return et[:].bitcast(F16)

        def emit_pv(hd, oT, k, et):
            for b in range(2):
                nc.tensor.matmul(
                    oT[:, b * 512:(b + 1) * 512],
                    hd.v1[:, k, :],
                    et[:, b * 512:(b + 1) * 512],
                    start=(k == 0), stop=(k == NQ - 1),
                )

        passes = [(h, qh) for h in range(HPC) for qh in range(S // QH)]
        s_carry = None
        for idx, (h, qh) in enumerate(passes):
            hd = heads[h]
            if qh == 0 and h + 1 < HPC:
                heads[h + 1].start_dma()

            oT = ps_o_pool.tile([D + 1, QH], F32, tag="oT", name="oT")
            pv_q = []
            for i in range(NPAIR):
                if i == 0 and s_carry is not None:
                    sA, sB = s_carry
                    s_carry = None
                else:
                    sA = ps_s_pool.tile([128, QH], F32, tag="s", name=f"sA{i}")
                    sB = ps_s_pool.tile([128, QH], F32, tag="s", name=f"sB{i}")
                    emit_scores_pair(hd, qh, i, sA, sB)
                etA = emit_exp(i, 2 * i, sA)
                etB = emit_exp(i, 2 * i + 1, sB)
                # pv emission lags one pair so the next scores pair outranks
                # pending pv in scheduler priority (measured ~2-3us better)
                pv_q.append((2 * i, etA))
                pv_q.append((2 * i + 1, etB))
                while len(pv_q) > 2:
                    k_, et_ = pv_q.pop(0)
                    emit_pv(hd, oT, k_, et_)
                del sA, sB, etA, etB
            for k_, et_ in pv_q:
                emit_pv(hd, oT, k_, et_)
            # hoist the next pass's first scores pair into this pass's tail
            if idx + 1 < len(passes):
                nh, nqh = passes[idx + 1]
                sA = ps_s_pool.tile([128, QH], F32, tag="s", name="scA")
                sB = ps_s_pool.tile([128, QH], F32, tag="s", name="scB")
                emit_scores_pair(heads[nh], nqh, 0, sA, sB)
                s_carry = (sA, sB)
            # DMA can't source PSUM: ACT (the PSUM-near engine) evicts to fp16
            o16 = epi_pool.tile([D + 1, QH], F16, tag="o16", name="o16")
            nc.scalar.copy(o16[:], oT[:])
            nc.sync.dma_start(oT_dram[h, qh], o16[:])


_NC_CACHE = None
_TRACE_READY = False


def _enable_tracing():
    """Register the NTFF profile hook that this image's antenv lacks, and
    keep profiling artifacts local instead of uploading to a bucket."""
    global _TRACE_READY
    if _TRACE_READY:
        return
    import sys
    import types

    import antenv
    import concourse.bass_utils as bu
    from trn_agent_boot.trn_boot import _ntff_profile_via_ctypes

    if "antenv.axon_hooks" not in sys.modules:
        mod = types.ModuleType("antenv.axon_hooks")
        mod._hook = None

        def set_axon_ntff_profile_hook(h):
            mod._hook = h

        def get_axon_ntff_profile_hook():
            return mod._hook

        mod.set_axon_ntff_profile_hook = set_axon_ntff_profile_hook
        mod.get_axon_ntff_profile_hook = get_axon_ntff_profile_hook
        sys.modules["antenv.axon_hooks"] = mod
        antenv.axon_hooks = mod

    hooks = sys.modules["antenv.axon_hooks"]
    if hooks.get_axon_ntff_profile_hook() is None:
        hooks.set_axon_ntff_profile_hook(
            _ntff_profile_via_ctypes("/opt/axon/libaxon_pjrt.so")
        )
    bu.upload_artifacts = lambda tmpdir: tmpdir
    _TRACE_READY = True


def _build():
    global _NC_CACHE
    if _NC_CACHE is None:
        nc = bacc.Bacc("TRN2", target_bir_lowering=False, debug=False)
        with tile.TileContext(nc) as tc:
            _attention(tc)
        nc.compile()
        _NC_CACHE = nc
    return _NC_CACHE


def _run(query, key, value, trace=False, tmpdir=None):
    if trace:
        _enable_tracing()
    q = np.asarray(query, dtype=np.float32).reshape(B * H, S, D)
    k = np.asarray(key, dtype=np.float32).reshape(B * H, S, D)
    v = np.asarray(value, dtype=np.float32).reshape(B * H, S, D)
    # host-side formatting: fp16 transposes + ones column
    qT = np.ascontiguousarray(q.transpose(0, 2, 1)).astype(np.float16)
    kT = np.ascontiguousarray(k.transpose(0, 2, 1)).astype(np.float16)
    v1 = np.empty((B * H, S, D + 1), dtype=np.float16)
    v1[:, :, :D] = v
    v1[:, :, D] = 1.0
    in_maps = [
        {
            "qT": qT[c * HPC:(c + 1) * HPC],
            "kT": kT[c * HPC:(c + 1) * HPC],
            "v1": v1[c * HPC:(c + 1) * HPC],
        }
        for c in range(N_CORES)
    ]
    nc = _build()
    res = run_bass_kernel_spmd(
        nc, in_maps, core_ids=list(range(N_CORES)), trace=trace, tmpdir=tmpdir
    )
    # host-side normalize + transpose: outT is [HPC, 2, D+1, QH] per core
    oT = np.stack([res.results[c]["outT"] for c in range(N_CORES)])
    oT = oT.reshape(B * H, 2, D + 1, QH).astype(np.float32)
    out = oT[:, :, :D, :] / (1.0 + oT[:, :, D:D + 1, :])
    out = out.transpose(0, 1, 3, 2).reshape(B, H, S, D)
    return np.ascontiguousarray(out), res


def kernel(query, key, value):
    out, _ = _run(query, key, value, trace=bool(int(os.environ.get("BASS_TRACE", "0"))))
    return out
